# revision 6
# baseline (speedup 1.0000x reference)
"""Trainium2 Bass kernel for Memorynet (KNN-interp + 1x1-conv MLP).

Strategy: pure data parallel over batch (32 batches -> 8 cores x 4).
Per batch, per 128-token tile:
  S = 2*p1@p2.T - |p2|^2  (one K=4 matmul into PSUM, [128 tok, 512 n2])
  top-8 via DVE max / max_index  (top-3 used)
  dist_k = |p1|^2 + eps - S_k ; w_k = (1/dist_k)/Z
  gather f2[idx_k] rows via indirect DMA  (token-major [128, 256])
  recvT (feature-major) accumulated in PSUM via  g_k.T @ diag(w_k)  matmuls
MLP is feature-major: out tiles = W.T chunks (lhsT) @ xT chunks (rhs);
BN+ReLU folded into ScalarE activation (per-partition scale/bias).
Host side handles all transposes / BN folding / sharding (numpy).
"""

import sys

sys.path.insert(0, "/opt/trn_rl_repo")

import numpy as np

import concourse.bass as bass
import concourse.bacc as bacc_mod
import concourse.mybir as mybir
from concourse.tile import TileContext
from concourse.masks import make_identity
from concourse.bass_utils import run_bass_kernel_spmd

EPS_DIST = 1e-8
EPS_BN = 1e-5
NCORES = 8
BPC = 4  # batches per core
N1, N2, C1, C2 = 2048, 512, 128, 256
CIN, H1, H2 = C1 + C2, 256, 128
NT = N1 // 128  # 16 token tiles / batch
GROUP = 4       # token tiles per MLP group (512 tokens)
NG = NT // GROUP

f32 = mybir.dt.float32
u32 = mybir.dt.uint32


def build_bass():
    nc = bacc_mod.Bacc()
    p1e = nc.declare_dram_parameter("p1e", [BPC, 4, N1], f32, isOutput=False)       # rows 0-2: p1T, row 3: ones
    rhs4 = nc.declare_dram_parameter("rhs4", [BPC, 4, N2], f32, isOutput=False)     # rows 0-2: 2*p2T, row3: -|p2|^2
    p1sqr = nc.declare_dram_parameter("p1sqr", [BPC, NT, 128, 8], f32, isOutput=False)  # |p1|^2+eps, rep 8
    f1T = nc.declare_dram_parameter("f1T", [BPC, C1, N1], f32, isOutput=False)
    f2s = [nc.declare_dram_parameter(f"f2_{b}", [N2, C2], f32, isOutput=False) for b in range(BPC)]
    W1Td = nc.declare_dram_parameter("W1T", [CIN, H1], f32, isOutput=False)
    W2Td = nc.declare_dram_parameter("W2T", [H1, H2], f32, isOutput=False)
    sb1d = nc.declare_dram_parameter("sb1", [H1, 2], f32, isOutput=False)
    sb2d = nc.declare_dram_parameter("sb2", [H2, 2], f32, isOutput=False)
    outT = nc.declare_dram_parameter("outT", [BPC, H2, N1], f32, isOutput=True)

    AT = mybir.ActivationFunctionType
    OP = mybir.AluOpType

    with TileContext(nc) as tc:
        with (
            tc.tile_pool(name="const", bufs=1) as cpool,
            tc.tile_pool(name="batch", bufs=2) as bpool,
            tc.tile_pool(name="grp", bufs=2) as gpool,
            tc.tile_pool(name="idxp", bufs=16) as idxpool,
            tc.tile_pool(name="gk", bufs=14) as gkpool,
            tc.tile_pool(name="diag", bufs=4) as dpool,
            tc.tile_pool(name="xg", bufs=2) as xpool,
            tc.tile_pool(name="ps_s", bufs=2, space="PSUM") as ps_s,
            tc.tile_pool(name="ps_recv", bufs=1, space="PSUM") as ps_recv,
            tc.tile_pool(name="ps_mlp", bufs=1, space="PSUM") as ps_mlp,
        ):
            # ---- constants ----
            ident = cpool.tile([128, 128], f32)
            make_identity(nc, ident[:])
            W1T = [cpool.tile([128, H1], f32, tag=f"w1_{k}", name=f"w1_{k}") for k in range(3)]
            for k in range(3):
                nc.sync.dma_start(out=W1T[k][:], in_=W1Td[128 * k:128 * (k + 1), :])
            W2T = [cpool.tile([128, H2], f32, tag=f"w2_{k}", name=f"w2_{k}") for k in range(2)]
            for k in range(2):
                nc.sync.dma_start(out=W2T[k][:], in_=W2Td[128 * k:128 * (k + 1), :])
            sb1 = [cpool.tile([128, 2], f32, tag=f"sb1_{k}", name=f"sb1_{k}") for k in range(2)]
            for k in range(2):
                nc.sync.dma_start(out=sb1[k][:], in_=sb1d[128 * k:128 * (k + 1), :])
            sb2 = cpool.tile([128, 2], f32)
            nc.sync.dma_start(out=sb2[:], in_=sb2d[:, :])

            for b in range(BPC):
                f1Tb = bpool.tile([C1, N1], f32, tag="f1Tb")
                nc.sync.dma_start(out=f1Tb[:], in_=f1T[b, :, :])
                p1eb = bpool.tile([4, N1], f32, tag="p1eb")
                nc.sync.dma_start(out=p1eb[:], in_=p1e[b, :, :])
                rhsb = bpool.tile([4, N2], f32, tag="rhsb")
                nc.sync.dma_start(out=rhsb[:], in_=rhs4[b, :, :])

                for g in range(NG):
                    p1sg = gpool.tile([128, GROUP, 8], f32, tag="p1sg")
                    nc.sync.dma_start(
                        out=p1sg[:],
                        in_=p1sqr[b, GROUP * g:GROUP * (g + 1), :, :].rearrange(
                            "t p k -> p t k"
                        ),
                    )
                    maxg = idxpool.tile([128, GROUP, 8], f32, tag="maxg")
                    idxg = idxpool.tile([128, GROUP, 8], u32, tag="idxg")
                    gks = []
                    for t in range(GROUP):
                        tau = GROUP * g + t
                        Sp = ps_s.tile([128, N2], f32, tag="Sp")
                        nc.tensor.matmul(
                            out=Sp[:],
                            lhsT=p1eb[:, 128 * tau:128 * (tau + 1)],
                            rhs=rhsb[:],
                            start=True,
                            stop=True,
                        )
                        nc.vector.max(out=maxg[:, t, :], in_=Sp[:])
                        nc.vector.max_index(
                            out=idxg[:, t, :], in_max=maxg[:, t, :], in_values=Sp[:]
                        )
                        for k in range(3):
                            gk = gkpool.tile([128, C2], f32, tag="gk")
                            nc.gpsimd.indirect_dma_start(
                                out=gk[:],
                                out_offset=None,
                                in_=f2s[b][:, :],
                                in_offset=bass.IndirectOffsetOnAxis(
                                    ap=idxg[:, t, k:k + 1], axis=0
                                ),
                            )
                            gks.append(gk)

                    # ---- group-batched weight math (FD=32 on DVE) ----
                    dist = gpool.tile([128, GROUP, 8], f32, tag="dist")
                    nc.vector.tensor_tensor(
                        out=dist[:], in0=p1sg[:], in1=maxg[:], op=OP.subtract
                    )
                    nc.vector.tensor_scalar_max(dist[:], dist[:], 1e-8)
                    recd = gpool.tile([128, GROUP, 8], f32, tag="recd")
                    nc.vector.reciprocal(out=recd[:], in_=dist[:])
                    Z = gpool.tile([128, GROUP], f32, tag="Z")
                    nc.vector.reduce_sum(
                        out=Z[:], in_=recd[:, :, 0:3], axis=mybir.AxisListType.X
                    )
                    Zinv = gpool.tile([128, GROUP], f32, tag="Zinv")
                    nc.vector.reciprocal(out=Zinv[:], in_=Z[:])
                    wg = gpool.tile([128, GROUP, 8], f32, tag="wg")
                    nc.vector.tensor_tensor(
                        out=wg[:],
                        in0=recd[:],
                        in1=Zinv[:, :, None].to_broadcast([128, GROUP, 8]),
                        op=OP.mult,
                    )

                    # ---- recvT accumulation: g_k.T @ diag(w_k) ----
                    recvp = [
                        ps_recv.tile([128, 512], f32, tag=f"recvp{h}", name=f"recvp{h}")
                        for h in range(2)
                    ]
                    for t in range(GROUP):
                        for k in range(3):
                            dk = dpool.tile([128, 128], f32, tag="diag")
                            nc.vector.tensor_scalar(
                                out=dk[:],
                                in0=ident[:],
                                scalar1=wg[:, t, k:k + 1],
                                scalar2=None,
                                op0=OP.mult,
                            )
                            gk = gks[3 * t + k]
                            for h in range(2):
                                nc.tensor.matmul(
                                    out=recvp[h][:, 128 * t:128 * (t + 1)],
                                    lhsT=gk[:, 128 * h:128 * (h + 1)],
                                    rhs=dk[:],
                                    start=(k == 0),
                                    stop=(k == 2),
                                )

                    # ---- xT chunks in SBUF: [recvT0, recvT1, f1T-slice] ----
                    xg = [xpool.tile([128, 512], f32, tag=f"xg{h}", name=f"xg{h}") for h in range(2)]
                    for h in range(2):
                        nc.scalar.activation(
                            out=xg[h][:], in_=recvp[h][:], func=AT.Copy, bias=0.0
                        )
                    f1sl = f1Tb[:, 512 * g:512 * (g + 1)]

                    # ---- L1: h1T [2x128, 512] ----
                    h1 = [xpool.tile([128, 512], f32, tag=f"h1_{m}", name=f"h1_{m}") for m in range(2)]
                    for m in range(2):
                        l1p = ps_mlp.tile([128, 512], f32, tag="l1p")
                        for kk in range(3):
                            rhs_kk = xg[kk][:] if kk < 2 else f1sl
                            nc.tensor.matmul(
                                out=l1p[:],
                                lhsT=W1T[kk][:, 128 * m:128 * (m + 1)],
                                rhs=rhs_kk,
                                start=(kk == 0),
                                stop=(kk == 2),
                            )
                        nc.scalar.activation(
                            out=h1[m][:],
                            in_=l1p[:],
                            func=AT.Relu,
                            scale=sb1[m][:, 0:1],
                            bias=sb1[m][:, 1:2],
                        )

                    # ---- L2: h2T [128, 512] ----
                    l2p = ps_mlp.tile([128, 512], f32, tag="l2p")
                    for kk in range(2):
                        nc.tensor.matmul(
                            out=l2p[:],
                            lhsT=W2T[kk][:],
                            rhs=h1[kk][:],
                            start=(kk == 0),
                            stop=(kk == 1),
                        )
                    o = xpool.tile([128, 512], f32, tag="osb")
                    nc.scalar.activation(
                        out=o[:],
                        in_=l2p[:],
                        func=AT.Relu,
                        scale=sb2[:, 0:1],
                        bias=sb2[:, 1:2],
                    )
                    nc.sync.dma_start(
                        out=outT[b, :, 512 * g:512 * (g + 1)], in_=o[:]
                    )
    nc.compile()
    return nc


_CACHE = {}


def _get_nc():
    if "nc" not in _CACHE:
        _CACHE["nc"] = build_bass()
    return _CACHE["nc"]


def _prep_core(inputs, c):
    """Host-side prep of one core's input map (batches 4c..4c+4)."""
    sl = slice(BPC * c, BPC * (c + 1))
    p1 = inputs["points_1"][sl]     # [4, N1, 3]
    p2 = inputs["points_2"][sl]     # [4, N2, 3]
    f1 = inputs["features_1"][sl]   # [4, N1, C1]
    f2 = inputs["features_2"][sl]   # [4, N2, C2]

    p1e = np.empty((BPC, 4, N1), np.float32)
    p1e[:, 0:3, :] = np.transpose(p1, (0, 2, 1))
    p1e[:, 3, :] = 1.0
    rhs4 = np.empty((BPC, 4, N2), np.float32)
    rhs4[:, 0:3, :] = 2.0 * np.transpose(p2, (0, 2, 1))
    rhs4[:, 3, :] = -np.sum(p2.astype(np.float64) ** 2, -1)
    p1sq = np.sum(p1.astype(np.float64) ** 2, -1) + EPS_DIST  # [4, N1]
    p1sqr = np.broadcast_to(
        p1sq.reshape(BPC, NT, 128, 1), (BPC, NT, 128, 8)
    ).astype(np.float32)
    m = {
        "p1e": np.ascontiguousarray(p1e),
        "rhs4": np.ascontiguousarray(rhs4.astype(np.float32)),
        "p1sqr": np.ascontiguousarray(p1sqr),
        "f1T": np.ascontiguousarray(np.transpose(f1, (0, 2, 1))),
    }
    for b in range(BPC):
        m[f"f2_{b}"] = np.ascontiguousarray(f2[b])
    # shared weights
    s1 = inputs["g1"] / np.sqrt(inputs["v1"] + EPS_BN)
    b1f = (inputs["b1"] - inputs["m1"]) * s1 + inputs["be1"]
    s2 = inputs["g2"] / np.sqrt(inputs["v2"] + EPS_BN)
    b2f = (inputs["b2"] - inputs["m2"]) * s2 + inputs["be2"]
    m["W1T"] = np.ascontiguousarray(inputs["W1"].T)
    m["W2T"] = np.ascontiguousarray(inputs["W2"].T)
    m["sb1"] = np.ascontiguousarray(np.stack([s1, b1f], -1).astype(np.float32))
    m["sb2"] = np.ascontiguousarray(np.stack([s2, b2f], -1).astype(np.float32))
    return m


def run(inputs, trace=False):
    nc = _get_nc()
    in_maps = [_prep_core(inputs, c) for c in range(NCORES)]
    res = run_bass_kernel_spmd(
        nc, in_maps, core_ids=list(range(NCORES)), trace=trace
    )
    outs = [np.asarray(r["outT"]) for r in res.results]
    full = np.concatenate(outs, 0)          # [32, H2, N1]
    out = np.ascontiguousarray(np.transpose(full, (0, 2, 1)))  # [32, N1, H2]
    return out, res


def kernel(**inputs):
    out, _ = run(inputs, trace=False)
    return out
